# revision 66
# baseline (speedup 1.0000x reference)
"""Causal self-attention Trainium2 kernel (B=2, T=2048, D=1024, H=16).

Sharding: 8 cores = 2 batch groups x 4 head groups; each core computes
batch b = c//4, heads 4*(c%4)..4*(c%4)+3 (256 QKV dims), and a partial
output projection y_cT = W_o[:, slice] @ attnout (contribution summed on
host across the 4 cores of each batch group).

Per-core pipeline (fp16 operands, fp32 PSUM accumulation):
  QT/KT = W @ xT                     [128, 2048] per head-pair p
  V     = x @ WvT                    [2048, 4*65] (seq on partitions,
                                      per-head 64 dims + ones column)
  ST[k,q] = sum_d K[k,d] Q[q,d]      (k on partitions, q streaming)
  P = exp(ST/8), diagonal 128x128 blocks masked
  av[q, 0:65] = sum_k P[k,q] [V|1]   <- streams only 65 rows per matmul;
                                        col 64 = softmax denominator
  attnout[q,d] = av * (1/denom)      (DVE reciprocal + per-partition mul)
  aoT = transpose(attnout)           (DMA XBAR transpose, SBUF->SBUF)
  yT[e,q] = WoT.T @ aoT              (partial over this core's 256 dims)

Schedule: x is DMA'd chunk-by-chunk and the block-0/1 projections
accumulate against the arriving stream; later-block projections and the
block-j output projections run as filler units inside attention so the
PE never starves while the scalar engine works through exp().  A few
exp chunks per late block run as Schraudolph bit-trick exponentials on
the vector/pool engines to unload the scalar engine.
"""

import numpy as np

import concourse.bass as bass
import concourse.mybir as mybir
from concourse.tile import TileContext
from concourse.vector_clock import ScopedClock
from concourse.bass_utils import run_bass_kernel_spmd

B, T, D = 2, 2048, 1024
H, DK = 16, 64
NCORES = 8
HPC = 4            # heads per core
QB = 512           # q block size
NQB = T // QB      # 4
NKC = T // 128     # 16 k-chunks
F16 = mybir.dt.float16
F32 = mybir.dt.float32
F8 = mybir.dt.float8e4
DRM = mybir.MatmulPerfMode.DoubleRow
EXPF = mybir.ActivationFunctionType.Exp


class TC(TileContext):
    """This container's walrus only accepts one sync-wait per TPB_CTRL
    instruction; split the tile tail-drain waits into one nop each."""

    def _drain_and_barrier(self, tick_clock, wait_clock):
        carrier = self.nc.sync.nop(nofuse=True)
        wait_clock.add_sem_waits(
            carrier.ins, ScopedClock({None: tick_clock.global_clock})
        )
        si = carrier.ins.sync_info
        if si is not None and len(si.on_wait) > 1:
            waits = list(si.on_wait)
            carrier.ins.sync_info = mybir.SyncInfo(
                on_wait=[waits[0]], on_update=list(si.on_update)
            )
            for w in waits[1:]:
                nop = self.nc.sync.nop(nofuse=True)
                nop.ins.sync_info = mybir.SyncInfo(on_wait=[w], on_update=[])
        self.nc.sync.drain()
        self.nc.all_engine_barrier()
        assert self.sems is not None
        popped = self.nc._tile_sem_poison_stack.pop()
        assert popped is self._sem_poison
        self.nc.clear_and_free_semaphores(list(self.sems.allocated().values()))
        self.nc.all_engine_barrier()


def split_multi_waits(nc):
    """This walrus build accepts only one sync-wait per instruction; hoist
    extra waits onto single-wait NoOps inserted just before the instruction
    on the same engine."""
    for fn in nc.m.functions:
        for bb in fn.blocks:
            out = []
            for ins in bb.instructions:
                si = getattr(ins, "sync_info", None)
                is_isa = "ISA" in type(ins).__name__ or "PartitionBroadcast" in type(ins).__name__
                keep = 0 if is_isa else 1
                if si is not None and len(si.on_wait) > keep:
                    waits = list(si.on_wait)
                    keep_waits = waits[len(waits) - keep :] if keep else []
                    for i, w in enumerate(waits[: len(waits) - keep]):
                        out.append(
                            mybir.InstNoOp(
                                name=f"{ins.name}_w{i}",
                                engine=ins.engine,
                                sync_info=mybir.SyncInfo(on_wait=[w], on_update=[]),
                                bass_nofuse=True,
                            )
                        )
                    ins.sync_info = mybir.SyncInfo(
                        on_wait=keep_waits, on_update=list(si.on_update)
                    )
                out.append(ins)
            bb.instructions = out


# Schraudolph fast-exp constants: exp(s/8) ~= bitcast_f32(int32(s*SCH_A + SCH_B))
SCH_C = 0.0579
SCH_A = float(2**23 * (np.log2(np.e) / 8.0 / 4096.0))
SCH_B = float(2**23 * (127.0 - SCH_C))
# (j, kc) -> engine for exp chunks offloaded off the scalar engine
SCH_CHUNKS = {
    (2, 1): "v", (2, 3): "p", (2, 5): "v",
    (3, 1): "v", (3, 5): "v", (3, 9): "v", (3, 3): "p", (3, 7): "p",
}


def build_nc():
    nc = bass.Bass("TRN2", target_bir_lowering=False, debug=False)
    # x in residual fp8 (hi + lo), contraction-chunk-pair major for DoubleRow:
    # col 4096*g + 2048*i + t = x[t, 128*(2g+i) + partition]
    x8d = nc.dram_tensor("x8", [128, 4 * 4096], F8, kind="ExternalInput")
    x8ld = nc.dram_tensor("x8l", [128, 4 * 4096], F8, kind="ExternalInput")
    # w8: [hi 6144 | lo 6144]; within each: wq | wk | wv blocks of 2048 cols,
    # DoubleRow pair-packed, weights pre-scaled by 64 (exp scale compensates)
    w8d = nc.dram_tensor("w8", [128, 2 * 6144], F8, kind="ExternalInput")
    # wom: [wo p0 | wo p1 | mask | identity]
    wom = nc.dram_tensor("wom", [128, 2 * D + 256], F16, kind="ExternalInput")
    yT = nc.dram_tensor("yT", [D, T], F16, kind="ExternalOutput")

    with TC(nc) as tc:
        with (
            tc.tile_pool(name="const", bufs=1) as cpool,
            tc.tile_pool(name="pP", bufs=2) as ppool,
            tc.tile_pool(name="work", bufs=2) as wpool,
            tc.tile_pool(name="psS", bufs=2, space="PSUM") as psS,
            tc.tile_pool(name="psAV", bufs=1, space="PSUM") as psAV,
            tc.tile_pool(name="psPR", bufs=2, space="PSUM") as psPR,
        ):
            # ---- loads: QKV weight pack first, then x (hi/lo fp8) pair
            # by pair so the block-0/1 projections stream against the DMA;
            # the output-projection pack (not needed until attention(2)) last
            w8s = cpool.tile([128, 2 * 6144], F8, tag="w8s", name="w8s")
            nc.sync.dma_start(w8s[:, 0:6144], w8d[:, 0:6144])
            x8t, x8lt = [], []
            for g in range(4):
                th = cpool.tile([128, 4096], F8, tag=f"x8t{g}", name=f"x8t{g}")
                nc.sync.dma_start(th[:], x8d[:, 4096 * g : 4096 * (g + 1)])
                x8t.append(th)
                tl = cpool.tile([128, 4096], F8, tag=f"x8lt{g}", name=f"x8lt{g}")
                nc.sync.dma_start(tl[:], x8ld[:, 4096 * g : 4096 * (g + 1)])
                x8lt.append(tl)
                if g == 0:
                    nc.sync.dma_start(w8s[:, 6144:12288], w8d[:, 6144:12288])

            def wqk8(res, woff, p, g):
                # lhsT [128, 2, 128] for Q/K m-group p, chunk pair g
                base = 6144 * res + woff + 1024 * p + 256 * g
                return w8s[:, base : base + 256].rearrange("p (i m) -> p i m", i=2)

            def wv8(res, g):
                # rhs [128, 2, 256] for the V projection, chunk pair g
                base = 6144 * res + 4096 + 512 * g
                return w8s[:, base : base + 512].rearrange("p (i n) -> p i n", i=2)

            def x8s(res, g, lo, hi):
                # rhs/lhsT [128, 2, hi-lo] of the x pair tiles
                t = x8lt[g] if res else x8t[g]
                return t[:].rearrange("p (i w) -> p i w", i=2)[:, :, lo:hi]
            woms = cpool.tile([128, 2 * D + 256], F16, tag="woms", name="woms")
            nc.sync.dma_start(woms[:], wom[:, :])
            wo = [woms[:, D * p : D * (p + 1)] for p in range(2)]
            mask = woms[:, 2 * D : 2 * D + 128]
            ident = woms[:, 2 * D + 128 : 2 * D + 256]

            qt = [cpool.tile([128, T], F16, tag=f"qt{p}", name=f"qt{p}") for p in range(2)]
            kt = [cpool.tile([128, T], F16, tag=f"kt{p}", name=f"kt{p}") for p in range(2)]
            ao = [cpool.tile([128, T], F16, tag=f"ao{p}", name=f"ao{p}") for p in range(2)]
            aoT = [cpool.tile([128, T], F16, tag=f"aoT{p}", name=f"aoT{p}") for p in range(2)]
            # V tiles: [128 seq, 4 heads x (64 dims + ones col)]; the ones
            # cols are memset once up front (overlapped with the x DMAs)
            vt = [cpool.tile([128, 260], F16, tag=f"vt{tt}", name=f"vt{tt}") for tt in range(NKC)]
            for tt in range(NKC):
                ones = vt[tt][:, 0:260].rearrange("p (h d) -> p h d", h=4)[:, :, 64:65]
                nc.gpsimd.memset(ones, 64.0)
            ysb = {}

            def get_ysb(j):
                if j not in ysb:
                    ysb[j] = wpool.tile([128, 8 * QB], F16, tag="ysb", name=f"ysb{j}", bufs=2)
                return ysb[j]

            # ---- projections (residual fp8 DoubleRow) ----
            RES3 = ((0, 0), (0, 1), (1, 0))  # (w_res, x_res): hi*hi, hi*lo, lo*hi

            def qk_proj(p, woff, out_t, jq):
                ps = psPR.tile([128, QB], F32, tag="pr", name=f"psqk{p}{jq}")
                for g in range(4):
                    for ri, (rw, rx) in enumerate(RES3):
                        nc.tensor.matmul(
                            ps[:],
                            wqk8(rw, woff, p, g),
                            x8s(rx, g, QB * jq, QB * (jq + 1)),
                            start=(g == 0 and ri == 0),
                            stop=(g == 3 and ri == 2),
                            perf_mode=DRM,
                        )
                nc.vector.tensor_copy(out_t[p][:, QB * jq : QB * (jq + 1)], ps[:])

            def v_finish(tt, ps):
                src = ps[:, 0:256].rearrange("p (h d) -> p h d", h=4)
                dst = vt[tt][:, 0:260].rearrange("p (h d) -> p h d", h=4)[:, :, 0:64]
                nc.vector.tensor_copy(dst, src)

            def v_proj(tt):
                ps = psPR.tile([128, QB], F32, tag="pr", name=f"psv{tt}")
                for g in range(4):
                    for ri, (rw, rx) in enumerate(RES3):
                        nc.tensor.matmul(
                            ps[:, 0:256],
                            x8s(rx, g, 128 * tt, 128 * (tt + 1)),
                            wv8(rw, g),
                            start=(g == 0 and ri == 0),
                            stop=(g == 3 and ri == 2),
                            perf_mode=DRM,
                        )
                v_finish(tt, ps)

            # ---- block-0/1 projections streamed against the x DMAs ----
            ps_q = psS.tile([128, 1024], F32, tag="psS", name="psq0")
            ps_k = psS.tile([128, 1024], F32, tag="psS", name="psk0")
            ps_v = [psPR.tile([128, QB], F32, tag="pr", name=f"psv{tt}") for tt in range(2)]
            ps_q1 = [psAV.tile([128, 512], F32, tag=t, name=f"psq1{p}") for p, t in ((0, "av0"), (1, "av1"))]
            for g in range(4):
                for ri, (rw, rx) in enumerate(RES3):
                    st, sp = (g == 0 and ri == 0), (g == 3 and ri == 2)
                    for p in range(2):
                        nc.tensor.matmul(
                            ps_q[:, 512 * p : 512 * (p + 1)],
                            wqk8(rw, 0, p, g),
                            x8s(rx, g, 0, QB),
                            start=st, stop=sp, perf_mode=DRM,
                        )
                    for p in range(2):
                        nc.tensor.matmul(
                            ps_k[:, 512 * p : 512 * (p + 1)],
                            wqk8(rw, 2048, p, g),
                            x8s(rx, g, 0, QB),
                            start=st, stop=sp, perf_mode=DRM,
                        )
                    for tt in range(2):
                        nc.tensor.matmul(
                            ps_v[tt][:, 0:256],
                            x8s(rx, g, 128 * tt, 128 * (tt + 1)),
                            wv8(rw, g),
                            start=st, stop=sp, perf_mode=DRM,
                        )
                    for p in range(2):
                        nc.tensor.matmul(
                            ps_q1[p][:],
                            wqk8(rw, 0, p, g),
                            x8s(rx, g, QB, 2 * QB),
                            start=st, stop=sp, perf_mode=DRM,
                        )
            for p in range(2):
                nc.vector.tensor_copy(qt[p][:, 0:QB], ps_q[:, 512 * p : 512 * (p + 1)])
                nc.vector.tensor_copy(kt[p][:, 0:QB], ps_k[:, 512 * p : 512 * (p + 1)])
                nc.vector.tensor_copy(qt[p][:, QB : 2 * QB], ps_q1[p][:])
            for tt in range(2):
                v_finish(tt, ps_v[tt])
            for tt in range(2, 4):
                v_proj(tt)

            # ---- out-projection for q block j (filler units) ----
            def o_proj_unit(j, et, copy_eng=None):
                po = psPR.tile([128, QB], F32, tag="pr", name=f"pso{j}{et}")
                for p in range(2):
                    nc.tensor.matmul(
                        po[:],
                        wo[p][:, 128 * et : 128 * (et + 1)],
                        aoT[p][:, QB * j : QB * (j + 1)],
                        start=(p == 0),
                        stop=(p == 1),
                    )
                eng = copy_eng or nc.vector
                if eng is nc.scalar:
                    eng.copy(get_ysb(j)[:, QB * et : QB * (et + 1)], po[:])
                else:
                    eng.tensor_copy(get_ysb(j)[:, QB * et : QB * (et + 1)], po[:])

            def o_store(j, lo, hi):
                nets = hi - lo
                nc.sync.dma_start(
                    yT[:, QB * j : QB * (j + 1)]
                    .rearrange("(c p) w -> p c w", p=128)[:, lo:hi, :],
                    get_ysb(j)[:, QB * lo : QB * hi].rearrange("p (c w) -> p c w", c=nets),
                )

            # ---- attention for q block j ----
            def attention(j, units=()):
                units = list(units)
                emitted = [0]
                nch = 4 * j + 4
                # pump steps: one per (p, kc) plus one per AV head-group so
                # high engine/seq-ratio filler hides the AV issue bubbles
                total_steps = 2 * (nch + 14)
                step = [0]

                def tick():
                    step[0] += 1
                    target = len(units) * step[0] // total_steps
                    while emitted[0] < target:
                        units[emitted[0]]()
                        emitted[0] += 1

                for p in range(2):
                    P = ppool.tile([128, 1024 * NKC], F16, tag="P", name=f"P{j}{p}")
                    for kc in range(nch):
                        qc = kc - 4 * j
                        off = max(0, 128 * qc)
                        ps = psS.tile([128, 1024], F32, tag="psS")
                        for h in range(2):
                            nc.tensor.matmul(
                                ps[:, 512 * h + off : 512 * (h + 1)],
                                kt[p][64 * h : 64 * (h + 1), 128 * kc : 128 * (kc + 1)],
                                qt[p][64 * h : 64 * (h + 1), QB * j + off : QB * (j + 1)],
                                start=True,
                                stop=True,
                            )
                        sch = SCH_CHUNKS.get((j, kc))
                        if sch is not None:
                            # fast exp unloads the scalar engine: the affine
                            # pass reads PSUM so it must run on DVE; the
                            # bitcast/convert pass can go to Pool
                            eng2 = nc.gpsimd
                            tmp = wpool.tile(
                                [128, 1024], mybir.dt.int32, tag=f"sch{sch}", bufs=2
                            )
                            nc.vector.tensor_scalar(
                                tmp[:], ps[:, 0:1024], SCH_A, SCH_B,
                                op0=mybir.AluOpType.mult, op1=mybir.AluOpType.add,
                            )
                            eng2.tensor_copy(
                                P[:, 1024 * kc : 1024 * (kc + 1)],
                                tmp[:].bitcast(F32),
                            )
                        else:
                            nc.scalar.activation(
                                P[:, 1024 * kc + off : 1024 * (kc + 1)],
                                ps[:, off:1024],
                                EXPF,
                                scale=0.125 / 4096.0,
                            )
                        if qc >= 0:  # diagonal band: mask 128x128 blocks
                            for h in range(2):
                                sl = slice(
                                    1024 * kc + 512 * h + off,
                                    1024 * kc + 512 * h + off + 128,
                                )
                                nc.vector.tensor_mul(P[:, sl], P[:, sl], mask[:])
                            # AV for q chunk qc: all needed k chunks are in P
                            for h in range(2):
                                hh = 2 * p + h
                                # full-bank tile so each accumulation group
                                # owns its own 2KB PSUM zero region
                                av = psAV.tile([128, 512], F32, tag="av0" if h == 0 else "av1")
                                for k2 in range(kc + 1):
                                    nc.tensor.matmul(
                                        av[:, 0:65],
                                        P[:, 1024 * k2 + 512 * h + 128 * qc : 1024 * k2 + 512 * h + 128 * qc + 128],
                                        vt[k2][:, 65 * hh : 65 * hh + 65],
                                        start=(k2 == 0),
                                        stop=(k2 == kc),
                                    )
                                tick()
                                rcp = wpool.tile([128, 1], F32, tag="rc", bufs=4)
                                nc.vector.reciprocal(rcp[:], av[:, 64:65])
                                nc.vector.tensor_scalar_mul(
                                    ao[p][:, QB * j + 128 * qc + 64 * h : QB * j + 128 * qc + 64 * h + 64],
                                    av[:, 0:64],
                                    rcp[:],
                                )
                            cols = slice(QB * j + 128 * qc, QB * j + 128 * (qc + 1))
                            if j == 3 and qc == 3:
                                # tail latency: PE transpose (+DVE copy) beats
                                # the XBAR DMA round trip by ~2us
                                pst = psAV.tile([128, 512], F32, tag="av0")
                                nc.tensor.transpose(
                                    pst[:, 0:64].bitcast(F16), ao[p][:, cols], ident
                                )
                                nc.vector.tensor_copy(
                                    aoT[p][:, cols], pst[:, 0:64].bitcast(F16)
                                )
                            elif j == 3:
                                # per-chunk XBAR transposes so the out-proj
                                # dependency clears as early as possible
                                nc.sync.dma_start(
                                    aoT[p][:, cols], ao[p][:, cols], transpose=True
                                )
                            elif qc == 3:
                                # one XBAR transpose [128,512] -> 4 blocks
                                nc.sync.dma_start(
                                    aoT[p][:, QB * j : QB * (j + 1)]
                                    .rearrange("p (c w) -> p c w", c=4),
                                    ao[p][:, QB * j : QB * (j + 1)],
                                    transpose=True,
                                )
                        tick()
                while step[0] < total_steps:
                    tick()

            # interleaved schedule: projections for block j+1 and the output
            # projections of earlier blocks run as filler inside attention(j)
            for j in range(NQB):
                units = []
                if j + 1 < NQB:
                    jq = j + 1
                    # Q(block 1) was already computed in the load stream
                    projs = ((0, 2048, kt), (1, 2048, kt)) if j == 0 else (
                        (0, 0, qt), (1, 0, qt), (0, 2048, kt), (1, 2048, kt))
                    for pp, wf, ot in projs:
                        units.append(lambda pp=pp, wf=wf, ot=ot, jq=jq: qk_proj(pp, wf, ot, jq))
                    for tt in range(4 * jq, 4 * jq + 4):
                        units.append(lambda tt=tt: v_proj(tt))
                if j == 3:
                    for jo in (0, 1, 2):
                        for et in range(8):
                            units.append(lambda jo=jo, et=et: o_proj_unit(jo, et, copy_eng=nc.scalar))
                        units.append(lambda jo=jo: o_store(jo, 0, 8))
                attention(j, units)
            # tail: block-3 output projection, stores split so only the last
            # small piece sits on the critical path
            for et in range(8):
                o_proj_unit(3, et, copy_eng=nc.vector if et >= 6 else None)
                if et == 3:
                    o_store(3, 0, 4)
                elif et == 5:
                    o_store(3, 4, 6)
                elif et == 6:
                    o_store(3, 6, 7)
            o_store(3, 7, 8)
    split_multi_waits(nc)
    return nc


_NC = None


def _get_nc():
    global _NC
    if _NC is None:
        _NC = build_nc()
    return _NC


def kernel(x, W_q, W_k, W_v, W_o):
    x = np.asarray(x, dtype=np.float32)
    W_q = np.asarray(W_q, dtype=np.float32)
    W_k = np.asarray(W_k, dtype=np.float32)
    W_v = np.asarray(W_v, dtype=np.float32)
    W_o = np.asarray(W_o, dtype=np.float32)

    import ml_dtypes

    E4 = ml_dtypes.float8_e4m3fn

    def q8(a):
        return a.astype(E4)

    tmask = np.triu(np.ones((128, 128), dtype=np.float16))
    ident = np.eye(128, dtype=np.float16)
    # x packs (shared by all cores of a batch group): residual fp8, chunk-pair
    # major: col 4096*g + 2048*i + t = x[t, 128*(2g+i) + partition]
    x8b, x8lb = [], []
    for b in range(B):
        xTf = np.ascontiguousarray(x[b].T)                 # [D, T] fp32
        xh = q8(xTf)
        xl = q8(xTf - xh.astype(np.float32))
        x8 = np.empty((128, 4 * 4096), dtype=E4)
        x8l = np.empty((128, 4 * 4096), dtype=E4)
        for g in range(4):
            for i in range(2):
                c = 2 * g + i
                x8[:, 4096 * g + 2048 * i : 4096 * g + 2048 * (i + 1)] = xh[128 * c : 128 * (c + 1), :]
                x8l[:, 4096 * g + 2048 * i : 4096 * g + 2048 * (i + 1)] = xl[128 * c : 128 * (c + 1), :]
        x8b.append(x8.view(np.uint8))
        x8lb.append(x8l.view(np.uint8))
    in_maps = []
    for c in range(NCORES):
        b, g = c // 4, c % 4
        hs = 256 * g
        # w8 pack: [hi | lo] x [wq | wk | wv], weights pre-scaled by 64;
        # q/k blocks are DoubleRow lhsT packs (p, pair, i, m), v is an rhs
        # pack (pair, i, n)
        w8 = np.empty((128, 2 * 6144), dtype=E4)
        for wi, W in enumerate((W_q, W_k, W_v)):
            wT = np.ascontiguousarray(W[hs : hs + 256, :].T) * 64.0  # [1024, 256]
            hi = q8(wT)
            lo = q8(wT - hi.astype(np.float32))
            for res, wr in ((0, hi), (1, lo)):
                base = 6144 * res + 2048 * wi
                if wi < 2:  # q/k: lhsT pack
                    for p in range(2):
                        for gg in range(4):
                            for i in range(2):
                                cc = 2 * gg + i
                                w8[:, base + 1024 * p + 256 * gg + 128 * i : base + 1024 * p + 256 * gg + 128 * (i + 1)] = wr[
                                    128 * cc : 128 * (cc + 1), 128 * p : 128 * (p + 1)
                                ]
                else:  # v: rhs pack
                    for gg in range(4):
                        for i in range(2):
                            cc = 2 * gg + i
                            w8[:, base + 512 * gg + 256 * i : base + 512 * gg + 256 * (i + 1)] = wr[
                                128 * cc : 128 * (cc + 1), :
                            ]
        # wom pack: wo (2 chunks of [128, 1024]) | mask | identity
        wom = np.empty((128, 2 * D + 256), dtype=np.float16)
        woT = W_o[:, hs : hs + 256].T  # [256, 1024]
        wom[:, 0:D] = woT[0:128, :]
        wom[:, D : 2 * D] = woT[128:256, :]
        wom[:, 2 * D : 2 * D + 128] = tmask
        wom[:, 2 * D + 128 : 2 * D + 256] = ident
        in_maps.append({"x8": x8b[b], "x8l": x8lb[b], "w8": w8.view(np.uint8), "wom": wom})
    res = run_bass_kernel_spmd(_get_nc(), in_maps, core_ids=list(range(NCORES)))
    out = np.empty((B, T, D), dtype=np.float32)
    for b in range(B):
        acc = res.results[4 * b]["yT"].astype(np.float32)
        for g in range(1, 4):
            acc = acc + res.results[4 * b + g]["yT"]
        out[b] = acc.T
    return out
